# revision 30
# baseline (speedup 1.0000x reference)
"""Trainium2 Bass kernel for nn_AttentionBlock (B=8, C=512, H=W=32, heads=8, groups=32).

Sharding: data-parallel over batch B across the 8 NeuronCores (1 batch element
per core, no collectives). Each core computes, for its X slice [512, 1024]:

    GroupNorm -> qkv 1x1 conv -> 8-head attention (S=1024, hd=64) -> proj -> +residual

v2 restructure: the kernel is paced by the ACT (scalar) engine's exp of the
8.4M attention scores — everything else hides under it.

  - PSUM is split statically: a 6-bank "ring" [128, 3072] holding 3 score
    regions of [128, 1024] each, plus a 2-bank "flex" tile [128, 1024] that
    alternates between attn@V accumulation, Q/K ride-along blobs, V^T psums
    and GroupNorm statistics (WAR-ordered by the tile framework).
  - scores^T per (pair, qn, kc) region: two row-tiled 64-contraction matmuls
    (heads 2p/2p+1 in PE rows 0-63/64-127).
  - exp runs in an alternating N=2048/N=1024 pattern over the ring (regions
    g,g+1 for g%3==0, region g for g%3==2), cutting ACT instruction overhead
    vs per-region exps. Output lands in a 24-plane bf16 SBUF ring.
  - attn@V for iteration it is emitted during iteration it+1 (2 matmuls per
    kc step) against [V | 1] blocks so PSUM row 64 accumulates the softmax
    denominator for free.
  - Q/K of later pairs are computed as 4-matmul "blobs" on the flex banks
    right after the previous iteration's attn@V is normalized out.
  - softmax denominators: DMA-spread over 128 partitions, fast reciprocal,
    DMA-broadcast, multiplied into attn@V output straight from PSUM.
  - proj reuses the ring (oc 0-2) + flex (oc 3) banks at the tail; bias +
    residual fused into the eviction; per-oc output DMA.
  - GroupNorm apply is split ACT/DVE; stats use DVE accumulate + ACT Square.
  - all matmuls bf16 with fp32 PSUM accumulation; GN statistics f32r.
"""
import numpy as np
import ml_dtypes
from contextlib import ExitStack

import concourse.bacc as bacc
import concourse.bass as bass
import concourse.tile as tile
from concourse import mybir
from concourse.bass_utils import run_bass_kernel_spmd

F32 = mybir.dt.float32
F32R = mybir.dt.float32r
BF16 = mybir.dt.bfloat16
FP8 = mybir.dt.float8e4
AF = mybir.ActivationFunctionType
AL = mybir.AluOpType

B, C, H, W = 8, 512, 32, 32
S = H * W            # 1024
NH = 8               # heads
HD = C // NH         # 64
NG = 32              # groups
GS = C // NG         # 16 channels per group
EPS = 1e-5
NCC = C // 128       # 4 channel chunks
NSC = S // 128       # 8 sequence chunks of 128
SCALE = HD ** -0.5   # 0.125
NIT = 8              # (pair, qn) iterations
NPL = 24             # exp sbuf ring planes
VHB = 80             # vT per-head block: 64 V + 1 ones + 15 pad (16B align)
EXPB = -2.0          # exp bias shift: keeps e^(x*scale-2) < 240 (fp8e4 max)


def build_nc():
    nc = bacc.Bacc("TRN2", target_bir_lowering=False, debug=False)

    # ---- DRAM parameters (per-core). Declaration order = binding order.
    x_d = nc.declare_dram_parameter("x", [C, S], F32, isOutput=False)
    qkvw_d = nc.declare_dram_parameter("qkv_wT", [C, 3 * C], BF16, isOutput=False)
    projw_d = nc.declare_dram_parameter("proj_wT", [C, C], BF16, isOutput=False)
    gsum_d = nc.declare_dram_parameter("gsum", [C, NG], F32R, isOutput=False)
    gexp_d = nc.declare_dram_parameter("gexpT", [NG, C], F32R, isOutput=False)
    w4_d = nc.declare_dram_parameter("norm_w4", [128, NCC], F32, isOutput=False)
    b4_d = nc.declare_dram_parameter("norm_b4", [128, NCC], F32, isOutput=False)
    qb_d = nc.declare_dram_parameter("qkv_b12", [128, 12], F32, isOutput=False)
    vb_d = nc.declare_dram_parameter("vb_bcast", [128, C], F32, isOutput=False)
    pb_d = nc.declare_dram_parameter("proj_b4", [128, NCC], F32, isOutput=False)
    y_d = nc.declare_dram_parameter("y", [C, S], F32, isOutput=True)

    # DRAM scratch for the softmax-denominator reciprocal broadcast.
    # layout [pair][qn][head-in-pair][q512]
    recip_d = nc.dram_tensor("recip_scratch", [NH // 2, 2, 2, 512], F32)

    with tile.TileContext(nc) as tc, ExitStack() as ctx:
        const = ctx.enter_context(tc.tile_pool(name="const", bufs=1))
        xp = ctx.enter_context(tc.tile_pool(name="xp", bufs=1))
        qp = ctx.enter_context(tc.tile_pool(name="qp", bufs=1))
        kp = ctx.enter_context(tc.tile_pool(name="kp", bufs=1))
        vp = ctx.enter_context(tc.tile_pool(name="vp", bufs=1))
        anp = ctx.enter_context(tc.tile_pool(name="anp", bufs=1))
        outp = ctx.enter_context(tc.tile_pool(name="outp", bufs=2))
        pwp = ctx.enter_context(tc.tile_pool(name="pwp", bufs=1))
        xnp = ctx.enter_context(tc.tile_pool(name="xnp", bufs=1))
        wqp = ctx.enter_context(tc.tile_pool(name="wqp", bufs=1))
        gnp = ctx.enter_context(tc.tile_pool(name="gnp", bufs=1))
        xep = ctx.enter_context(tc.tile_pool(name="xep", bufs=NPL))
        rcp = ctx.enter_context(tc.tile_pool(name="rcp", bufs=4))
        xsqp = ctx.enter_context(tc.tile_pool(name="xsqp", bufs=2))
        ring_pool = ctx.enter_context(
            tc.tile_pool(name="ring_pool", bufs=1, space="PSUM"))
        flex_pool = ctx.enter_context(
            tc.tile_pool(name="flex_pool", bufs=1, space="PSUM"))

        # ---------- input / weight DMAs (stats-critical first) ----------
        x_sb = [xp.tile([128, S], F32, tag=f"x{cc}", name=f"x{cc}")
                for cc in range(NCC)]
        for cc in range(NCC):
            nc.sync.dma_start(x_sb[cc][:], x_d[128 * cc:128 * (cc + 1), :])
        gsum_sb = gnp.tile([C // NCC, NG * NCC], F32R)
        for cc in range(NCC):
            nc.sync.dma_start(gsum_sb[:, NG * cc:NG * (cc + 1)],
                              gsum_d[128 * cc:128 * (cc + 1), :])
        w4_sb = const.tile([128, NCC], F32)
        nc.sync.dma_start(w4_sb[:], w4_d[:])
        b4_sb = const.tile([128, NCC], F32)
        nc.sync.dma_start(b4_sb[:], b4_d[:])
        gexp_sb = const.tile([NG, C], F32R)
        nc.sync.dma_start(gexp_sb[:], gexp_d[:])
        qb_sb = const.tile([128, 12], F32)
        nc.sync.dma_start(qb_sb[:], qb_d[:])

        qkvw_sb = [wqp.tile([128, 3 * C], BF16, tag=f"w{cc}", name=f"w{cc}")
                   for cc in range(NCC)]
        for cc in range(NCC):
            nc.sync.dma_start(qkvw_sb[cc][:],
                              qkvw_d[128 * cc:128 * (cc + 1), :])
        vb_sb = const.tile([128, C], F32)
        nc.sync.dma_start(vb_sb[:], vb_d[:])
        pb_sb = const.tile([128, NCC], F32)
        nc.sync.dma_start(pb_sb[:], pb_d[:])
        pw_sb = [pwp.tile([128, C], BF16, tag=f"pw{cc}", name=f"pw{cc}")
                 for cc in range(NCC)]
        for cc in range(NCC):
            nc.sync.dma_start(pw_sb[cc][:], projw_d[128 * cc:128 * (cc + 1), :])

        # ---------- SBUF working tiles ----------
        q_sb = [qp.tile([128, S], BF16, tag=f"q{cc}", name=f"q{cc}")
                for cc in range(NH // 2)]
        k_sb = [kp.tile([128, S], BF16, tag=f"k{cc}", name=f"k{cc}")
                for cc in range(NH // 2)]
        # [64 v-channels | 1.0 | pad] per head block (fp8, DoubleRow layout
        # [key%128, kc, head*VHB + ch]): the ones column turns the attn@V
        # matmul (M=65) into attn@V plus the softmax denominator row.
        vt3 = vp.tile([128, NSC, VHB * NH], BF16, tag="vt3")
        an_sb = [anp.tile([128, S], BF16, tag=f"an{cc}", name=f"an{cc}")
                 for cc in range(NCC)]
        xn_sb = [xnp.tile([128, S], BF16, tag=f"xn{cc}", name=f"xn{cc}")
                 for cc in range(NCC)]
        vt3h = vt3[:].rearrange("p s (h u) -> p s h u", u=VHB)
        nc.vector.memset(vt3h[:, :, :, 64:65], 1.0)
        expb_sb = const.tile([128, 1], F32)
        nc.vector.memset(expb_sb[:], EXPB)

        # PSUM: 6-bank score ring (3 rotating region tiles) + 2-bank flex.
        flex = flex_pool.tile([128, S], F32, tag="flex")

        # load the ln/exp ACT table set while the input DMAs run
        warm = gnp.tile([1, 1], F32)
        nc.vector.memset(warm[:], 1.0)
        nc.scalar.activation(out=warm[:], in_=warm[:], func=AF.Ln,
                             bias=warm[:], scale=1.0)

        # ================= GroupNorm ================
        # per-channel sum (DVE accumulate) and sum of squares (ACT Square
        # accumulate); a tiny f32r matmul against the group map then does
        # the cross-partition group reduction.
        s12 = gnp.tile([128, 2 * NCC], F32)
        for cc in range(NCC):
            scr = xsqp.tile([128, S], BF16, tag="scr")
            nc.vector.scalar_tensor_tensor(
                out=scr[:], in0=x_sb[cc][:], scalar=1.0, in1=x_sb[cc][:],
                op0=AL.mult, op1=AL.bypass,
                accum_out=s12[:, 2 * cc:2 * cc + 1])
            scr2 = xsqp.tile([128, S], BF16, tag="scr2")
            nc.scalar.activation(
                out=scr2[:], in_=x_sb[cc][:], func=AF.Square,
                accum_out=s12[:, 2 * cc + 1:2 * cc + 2])
        s12r = gnp.tile([128, 2 * NCC], F32R)
        nc.vector.tensor_copy(s12r[:], s12[:])
        ps_g = flex[0:NG, 0:2]
        for cc in range(NCC):
            nc.tensor.matmul(
                ps_g, gsum_sb[:, NG * cc:NG * (cc + 1)],
                s12r[:, 2 * cc:2 * cc + 2],
                start=(cc == 0), stop=(cc == NCC - 1))
        inv_n = 1.0 / (GS * S)
        mean_g = gnp.tile([NG, 1], F32)
        nc.vector.tensor_scalar(out=mean_g[:], in0=ps_g[:, 0:1],
                                scalar1=inv_n,
                                scalar2=None, op0=AL.mult)
        ex2 = gnp.tile([NG, 1], F32)
        nc.vector.tensor_scalar(out=ex2[:], in0=ps_g[:, 1:2],
                                scalar1=inv_n,
                                scalar2=None, op0=AL.mult)
        var_g = gnp.tile([NG, 1], F32)
        # var = E[x^2] - mean^2
        nc.vector.scalar_tensor_tensor(
            out=var_g[:], in0=mean_g[:], scalar=-1.0, in1=mean_g[:],
            op0=AL.mult, op1=AL.mult)
        nc.vector.tensor_tensor(out=var_g[:], in0=ex2[:], in1=var_g[:],
                                op=AL.add)
        # rstd = exp(-0.5 * ln(var + eps)); ln+exp share one ACT table set
        eps_sb = gnp.tile([NG, 1], F32)
        nc.vector.memset(eps_sb[:], EPS)
        lnv = gnp.tile([NG, 1], F32)
        nc.scalar.activation(out=lnv[:], in_=var_g[:], func=AF.Ln,
                             bias=eps_sb[:], scale=1.0)
        # stats_r[:, 0] = rstd, stats_r[:, 1] = mean  (N=2 matmul rhs)
        stats_r = gnp.tile([NG, 2], F32R)
        nc.scalar.activation(out=stats_r[:, 0:1], in_=lnv[:], func=AF.Exp,
                             bias=0.0, scale=-0.5)
        nc.vector.tensor_copy(stats_r[:, 1:2], mean_g[:])

        # per-channel rstd/mean via tiny matmuls against the group map
        rstd_c = gnp.tile([128, NCC], F32)
        mean_c = gnp.tile([128, NCC], F32)
        for cc in range(NCC):
            ps_a = flex[:, 4 + 2 * cc:6 + 2 * cc]
            nc.tensor.matmul(ps_a,
                             gexp_sb[:, 128 * cc:128 * (cc + 1)],
                             stats_r[:], start=True, stop=True)
            nc.vector.tensor_copy(rstd_c[:, cc:cc + 1], ps_a[:, 0:1])
            nc.vector.tensor_copy(mean_c[:, cc:cc + 1], ps_a[:, 1:2])
        alpha = gnp.tile([128, NCC], F32)
        nc.vector.tensor_tensor(out=alpha[:], in0=rstd_c[:], in1=w4_sb[:],
                                op=AL.mult)
        beta = gnp.tile([128, NCC], F32)
        nc.vector.tensor_tensor(out=beta[:], in0=alpha[:], in1=mean_c[:],
                                op=AL.mult)
        nc.vector.tensor_tensor(out=beta[:], in0=b4_sb[:], in1=beta[:],
                                op=AL.subtract)

        # ---------- GN apply: split ACT / DVE ----------
        for cc in range(NCC):
            if cc < 2:
                nc.scalar.activation(
                    out=xn_sb[cc][:], in_=x_sb[cc][:], func=AF.Identity,
                    bias=beta[:, cc:cc + 1], scale=alpha[:, cc:cc + 1])
            else:
                nc.vector.tensor_scalar(
                    out=xn_sb[cc][:], in0=x_sb[cc][:],
                    scalar1=alpha[:, cc:cc + 1], scalar2=beta[:, cc:cc + 1],
                    op0=AL.mult, op1=AL.add)

        # ---------- Q0 / K0 (in ring-pool rotations, evicted pre-scores) --
        for dst, woff, boff in ((q_sb[0], 0, 0), (k_sb[0], 512, 4)):
            ps_qk = ring_pool.tile([128, S], F32, tag="sc", name="ps_qk")
            for cc in range(NCC):
                for hq in range(2):
                    nc.tensor.matmul(ps_qk[:, 512 * hq:512 * (hq + 1)],
                                     qkvw_sb[cc][:, woff:woff + 128],
                                     xn_sb[cc][:, 512 * hq:512 * (hq + 1)],
                                     start=(cc == 0), stop=(cc == NCC - 1))
            nc.vector.tensor_scalar(out=dst[:], in0=ps_qk[:],
                                    scalar1=qb_sb[:, boff:boff + 1],
                                    scalar2=None, op0=AL.add)

        # ================= attention ================
        # Emission helpers. Iteration it = (pair p = it>>1, qn = it&1).
        sc_tiles = {}
        ep_tiles = {}

        def emit_scores(it, kc):
            p, qn = it >> 1, it & 1
            g = 8 * it + kc
            sc_t = ring_pool.tile([128, S], F32, tag="sc", name=f"sc{g}")
            sc_tiles[g] = sc_t
            nc.tensor.matmul(
                sc_t[:, 0:512],
                k_sb[p][0:64, 128 * kc:128 * (kc + 1)],
                q_sb[p][0:64, 512 * qn:512 * (qn + 1)],
                start=True, stop=True, tile_position=(0, 0))
            nc.tensor.matmul(
                sc_t[:, 512:1024],
                k_sb[p][64:128, 128 * kc:128 * (kc + 1)],
                q_sb[p][64:128, 512 * qn:512 * (qn + 1)],
                start=True, stop=True, tile_position=(64, 0))

        def emit_exps(it, kc):
            g = 8 * it + kc
            ep_t = xep.tile([128, S], BF16, tag="ep", name=f"ep{g}")
            ep_tiles[g] = ep_t
            nc.scalar.activation(out=ep_t[:],
                                 in_=sc_tiles.pop(g)[:],
                                 func=AF.Exp, bias=expb_sb[:], scale=SCALE)

        def emit_attnv(av, it, kc):
            p = it >> 1
            g = 8 * it + kc
            ep_t = ep_tiles.pop(g)
            for h in range(2):
                hh = 2 * p + h
                nc.tensor.matmul(
                    av[0:65, 512 * h:512 * (h + 1)],
                    vt3[:, kc, VHB * hh:VHB * hh + 65],
                    ep_t[:, 512 * h:512 * (h + 1)],
                    start=(kc == 0), stop=(kc == NSC - 1))

        def emit_vt(sc):
            psv = flex[:, 512 * (sc % 2):512 * (sc % 2) + 512]
            for cc in range(NCC):
                nc.tensor.matmul(
                    psv,
                    xn_sb[cc][:, 128 * sc:128 * (sc + 1)],
                    qkvw_sb[cc][:, 1024:1536],
                    start=(cc == 0), stop=(cc == NCC - 1))
            nc.vector.tensor_tensor(
                out=vt3h[:, sc, :, 0:64],
                in0=psv.rearrange("p (h u) -> p h u", u=64),
                in1=vb_sb[:].rearrange("p (h u) -> p h u", u=64),
                op=AL.add)

        def emit_blob(tgt_pair, is_k):
            woff = 512 + 128 * tgt_pair if is_k else 128 * tgt_pair
            for cc in range(NCC):
                for hq in range(2):
                    nc.tensor.matmul(flex[:, 512 * hq:512 * (hq + 1)],
                                     qkvw_sb[cc][:, woff:woff + 128],
                                     xn_sb[cc][:, 512 * hq:512 * (hq + 1)],
                                     start=(cc == 0), stop=(cc == NCC - 1))
            dst = k_sb[tgt_pair] if is_k else q_sb[tgt_pair]
            boff = 4 + tgt_pair if is_k else tgt_pair
            nc.vector.tensor_scalar(out=dst[:], in0=flex[:, :],
                                    scalar1=qb_sb[:, boff:boff + 1],
                                    scalar2=None, op0=AL.add)

        # finish is split: emit_recip evicts attn@V + launches the
        # denominator-reciprocal DMA round trip; emit_norm (emitted two
        # iterations later, once the broadcast has surely landed) does the
        # softmax-normalize multiplies. This keeps the DMA latency off the
        # in-order DVE/PE queues.
        norm_state = {}

        def emit_recip(av, it):
            p, qn = it >> 1, it & 1
            raw = rcp.tile([65, S], F32, tag="raw")
            nc.vector.tensor_copy(raw[:], av[0:65, :])
            d128 = rcp.tile([128, 8], F32, tag="d128")
            nc.sync.dma_start(d128[:], raw[64:65, :])
            r128 = rcp.tile([128, 8], F32, tag="r128")
            rscr = rcp.tile([128, 8], F32, tag="rscr")
            nc.vector.reciprocal_approx_accurate(
                out=r128[:], in_=d128[:], scratch=rscr[:])
            r128v = recip_d[p][qn].rearrange("h (x f) -> (h x) f", f=8)
            nc.sync.dma_start(r128v, r128[:])
            rb = rcp.tile([64, S], F32, tag="rb")
            rsrc = recip_d[p][qn].rearrange("h f -> (h f)")  # [1024]
            rsrc_b = bass.AP(tensor=rsrc.tensor,
                             offset=rsrc.offset,
                             ap=[[0, 64], list(rsrc.ap[0])])
            nc.sync.dma_start(rb[:], rsrc_b)
            norm_state[it] = (raw, rb)

        def emit_norm(it):
            p, qn = it >> 1, it & 1
            raw, rb = norm_state.pop(it)
            nc.vector.tensor_tensor(
                out=an_sb[p][0:64, 512 * qn:512 * (qn + 1)],
                in0=raw[0:64, 0:512], in1=rb[:, 0:512],
                op=AL.mult)
            nc.vector.tensor_tensor(
                out=an_sb[p][64:128, 512 * qn:512 * (qn + 1)],
                in0=raw[0:64, 512:1024], in1=rb[:, 512:1024],
                op=AL.mult)

        # blobs delivered after finishing iteration it: [(target pair, is_k)]
        # finish(2P-2) is emitted at the end of iteration 2P-1, just before
        # pair P's scores start in iteration 2P.
        blob_after = {0: [(1, False), (1, True)],
                      2: [(2, False), (2, True)],
                      4: [(3, False), (3, True)]}


        av_tiles = {}
        for it in range(NIT):
            if it > 0:
                av_tiles[it - 1] = flex[:, :]
            for kc in range(NSC):
                emit_scores(it, kc)
                emit_exps(it, kc)
                if it == 0:
                    # V^T rides in iteration 0 on the flex banks
                    emit_vt(kc)
                else:
                    emit_attnv(av_tiles[it - 1], it - 1, kc)
            if it == 0:
                continue
            # finish iteration it-1: denominators, normalize, then the
            # flex banks host the next Q/K blob.
            emit_recip(av_tiles[it - 1], it - 1)
            for blob in blob_after.get(it - 1, ()):
                emit_blob(*blob)
            if it - 3 >= 0:
                emit_norm(it - 3)
        # last iteration's attn@V (runs as its exps land), then proj
        # partials for the already-normalized pairs overlap the recip tail.
        av_tiles[NIT - 1] = flex[:, :]
        for kc in range(NSC):
            emit_attnv(av_tiles[NIT - 1], NIT - 1, kc)
        emit_recip(av_tiles[NIT - 1], NIT - 1)
        emit_norm(NIT - 3)
        # ================= proj + bias + residual ================
        # oc 0-2 in ring-pool rotations (banks free as the last exps drain),
        # oc 3 on flex (frees after the last raw eviction).
        proj_ps = [ring_pool.tile([128, S], F32, tag="sc", name=f"pso{oc}")
                   for oc in range(3)] + [flex[:, :]]
        # partials over the already-normalized pairs (cc 0-2) overlap the
        # last reciprocal's DMA round trip; oc3 (flex) starts once the raw
        # eviction has freed the flex banks.
        for cc in range(3):
            for oc in range(NCC):
                tgt = proj_ps[oc]
                for hq in range(2):
                    nc.tensor.matmul(
                        tgt[:, 512 * hq:512 * (hq + 1)],
                        pw_sb[cc][:, 128 * oc:128 * (oc + 1)],
                        an_sb[cc][:, 512 * hq:512 * (hq + 1)],
                        start=(cc == 0), stop=False)
        emit_norm(NIT - 2)
        emit_norm(NIT - 1)
        for oc in range(NCC):
            for hq in range(2):
                nc.tensor.matmul(
                    proj_ps[oc][:, 512 * hq:512 * (hq + 1)],
                    pw_sb[3][:, 128 * oc:128 * (oc + 1)],
                    an_sb[3][:, 512 * hq:512 * (hq + 1)],
                    start=False, stop=True)
        ps_o = proj_ps
        for oc in range(NCC):
            out_t = outp.tile([128, S], F32, tag="out")
            if oc < 2:
                nc.vector.scalar_tensor_tensor(
                    out=out_t[:], in0=ps_o[oc],
                    scalar=pb_sb[:, oc:oc + 1], in1=x_sb[oc][:],
                    op0=AL.add, op1=AL.add)
            else:
                tmp_t = outp.tile([128, S], F32, tag="tmp")
                nc.scalar.activation(out=tmp_t[:], in_=ps_o[oc],
                                     func=AF.Identity,
                                     bias=pb_sb[:, oc:oc + 1], scale=1.0)
                nc.vector.tensor_tensor(out=out_t[:], in0=tmp_t[:],
                                        in1=x_sb[oc][:],
                                        op=AL.add)
            nc.sync.dma_start(y_d[128 * oc:128 * (oc + 1), :], out_t[:])

    nc.finalize()
    return nc


_NC_CACHE = None


def _get_nc():
    global _NC_CACHE
    if _NC_CACHE is None:
        _NC_CACHE = build_nc()
    return _NC_CACHE


def make_in_maps(X, norm_w, norm_b, qkv_w, qkv_b, proj_w, proj_b):
    X = np.asarray(X, dtype=np.float32)
    norm_w = np.asarray(norm_w, dtype=np.float32)
    norm_b = np.asarray(norm_b, dtype=np.float32)
    qkv_w = np.asarray(qkv_w, dtype=np.float32)
    qkv_b = np.asarray(qkv_b, dtype=np.float32)
    proj_w = np.asarray(proj_w, dtype=np.float32)
    proj_b = np.asarray(proj_b, dtype=np.float32)

    qkv_wT = np.ascontiguousarray(qkv_w.T).astype(ml_dtypes.bfloat16)
    proj_wT = np.ascontiguousarray(proj_w.T).astype(ml_dtypes.bfloat16)
    gsum = np.zeros((C, NG), np.float32)
    gsum[np.arange(C), np.arange(C) // GS] = 1.0
    gexpT = np.ascontiguousarray(gsum.T)                      # [32, 512]
    w4 = np.ascontiguousarray(norm_w.reshape(NCC, 128).T)     # [128, 4]
    b4 = np.ascontiguousarray(norm_b.reshape(NCC, 128).T)
    qb12 = np.ascontiguousarray(qkv_b.reshape(12, 128).T)     # [128, 12]
    vb_bcast = np.ascontiguousarray(
        np.broadcast_to(qkv_b[2 * C:3 * C], (128, C)))        # [128, 512]
    pb4 = np.ascontiguousarray(proj_b.reshape(NCC, 128).T)

    shared = {
        "qkv_wT": qkv_wT, "proj_wT": proj_wT, "gsum": gsum, "gexpT": gexpT,
        "norm_w4": w4, "norm_b4": b4, "qkv_b12": qb12, "vb_bcast": vb_bcast,
        "proj_b4": pb4,
    }
    in_maps = []
    for b in range(B):
        m = dict(shared)
        m["x"] = np.ascontiguousarray(X[b].reshape(C, S))
        in_maps.append(m)
    return in_maps


def kernel(X, norm_w, norm_b, qkv_w, qkv_b, proj_w, proj_b):
    nc = _get_nc()
    in_maps = make_in_maps(X, norm_w, norm_b, qkv_w, qkv_b, proj_w, proj_b)
    res = run_bass_kernel_spmd(nc, in_maps, core_ids=list(range(B)))
    out = np.stack([res.results[b]["y"].reshape(C, H, W) for b in range(B)])
    return out.astype(np.float32)


# revision 31
# speedup vs baseline: 1.1706x; 1.1706x over previous
"""Trainium2 Bass kernel for nn_AttentionBlock (B=8, C=512, H=W=32, heads=8, groups=32).

Sharding: data-parallel over batch B across the 8 NeuronCores (1 batch element
per core, no collectives). Each core computes, for its X slice [512, 1024]:

    GroupNorm -> qkv 1x1 conv -> 8-head attention (S=1024, hd=64) -> proj -> +residual

v2 restructure: the kernel is paced by the ACT (scalar) engine's exp of the
8.4M attention scores — everything else hides under it.

  - PSUM is split statically: a 6-bank "ring" [128, 3072] holding 3 score
    regions of [128, 1024] each, plus a 2-bank "flex" tile [128, 1024] that
    alternates between attn@V accumulation, Q/K ride-along blobs, V^T psums
    and GroupNorm statistics (WAR-ordered by the tile framework).
  - scores^T per (pair, qn, kc) region: two row-tiled 64-contraction matmuls
    (heads 2p/2p+1 in PE rows 0-63/64-127).
  - exp runs in an alternating N=2048/N=1024 pattern over the ring (regions
    g,g+1 for g%3==0, region g for g%3==2), cutting ACT instruction overhead
    vs per-region exps. Output lands in a 24-plane bf16 SBUF ring.
  - attn@V for iteration it is emitted during iteration it+1 (2 matmuls per
    kc step) against [V | 1] blocks so PSUM row 64 accumulates the softmax
    denominator for free.
  - Q/K of later pairs are computed as 4-matmul "blobs" on the flex banks
    right after the previous iteration's attn@V is normalized out.
  - softmax denominators: DMA-spread over 128 partitions, fast reciprocal,
    DMA-broadcast, multiplied into attn@V output straight from PSUM.
  - proj reuses the ring (oc 0-2) + flex (oc 3) banks at the tail; bias +
    residual fused into the eviction; per-oc output DMA.
  - GroupNorm apply is split ACT/DVE; stats use DVE accumulate + ACT Square.
  - all matmuls bf16 with fp32 PSUM accumulation; GN statistics f32r.
"""
import numpy as np
import ml_dtypes
from contextlib import ExitStack

import concourse.bacc as bacc
import concourse.bass as bass
import concourse.tile as tile
from concourse import mybir
from concourse.bass_utils import run_bass_kernel_spmd

F32 = mybir.dt.float32
F32R = mybir.dt.float32r
BF16 = mybir.dt.bfloat16
FP8 = mybir.dt.float8e4
AF = mybir.ActivationFunctionType
AL = mybir.AluOpType

B, C, H, W = 8, 512, 32, 32
S = H * W            # 1024
NH = 8               # heads
HD = C // NH         # 64
NG = 32              # groups
GS = C // NG         # 16 channels per group
EPS = 1e-5
NCC = C // 128       # 4 channel chunks
NSC = S // 128       # 8 sequence chunks of 128
SCALE = HD ** -0.5   # 0.125
NIT = 8              # (pair, qn) iterations
NPL = 24             # exp sbuf ring planes
VHB = 80             # vT per-head block: 64 V + 1 ones + 15 pad (16B align)
EXPB = -2.0          # exp bias shift: keeps e^(x*scale-2) < 240 (fp8e4 max)


def build_nc():
    nc = bacc.Bacc("TRN2", target_bir_lowering=False, debug=False)

    # ---- DRAM parameters (per-core). Declaration order = binding order.
    x_d = nc.declare_dram_parameter("x", [C, S], F32, isOutput=False)
    qkvw_d = nc.declare_dram_parameter("qkv_wT", [C, 3 * C], BF16, isOutput=False)
    projw_d = nc.declare_dram_parameter("proj_wT", [C, C], BF16, isOutput=False)
    gsum_d = nc.declare_dram_parameter("gsum", [C, NG], F32R, isOutput=False)
    gexp_d = nc.declare_dram_parameter("gexpT", [NG, C], F32R, isOutput=False)
    w4_d = nc.declare_dram_parameter("norm_w4", [128, NCC], F32, isOutput=False)
    b4_d = nc.declare_dram_parameter("norm_b4", [128, NCC], F32, isOutput=False)
    qb_d = nc.declare_dram_parameter("qkv_b12", [128, 12], F32, isOutput=False)
    vb_d = nc.declare_dram_parameter("vb_bcast", [128, C], F32, isOutput=False)
    pb_d = nc.declare_dram_parameter("proj_b4", [128, NCC], F32, isOutput=False)
    y_d = nc.declare_dram_parameter("y", [C, S], F32, isOutput=True)

    # DRAM scratch for the softmax-denominator reciprocal broadcast.
    # layout [pair][qn][head-in-pair][q512]
    recip_d = nc.dram_tensor("recip_scratch", [NH // 2, 2, 2, 512], F32)

    with tile.TileContext(nc) as tc, ExitStack() as ctx:
        const = ctx.enter_context(tc.tile_pool(name="const", bufs=1))
        xp = ctx.enter_context(tc.tile_pool(name="xp", bufs=1))
        qp = ctx.enter_context(tc.tile_pool(name="qp", bufs=1))
        kp = ctx.enter_context(tc.tile_pool(name="kp", bufs=1))
        vp = ctx.enter_context(tc.tile_pool(name="vp", bufs=1))
        anp = ctx.enter_context(tc.tile_pool(name="anp", bufs=1))
        outp = ctx.enter_context(tc.tile_pool(name="outp", bufs=2))
        pwp = ctx.enter_context(tc.tile_pool(name="pwp", bufs=1))
        xnp = ctx.enter_context(tc.tile_pool(name="xnp", bufs=1))
        wqp = ctx.enter_context(tc.tile_pool(name="wqp", bufs=1))
        gnp = ctx.enter_context(tc.tile_pool(name="gnp", bufs=1))
        xep = ctx.enter_context(tc.tile_pool(name="xep", bufs=NPL))
        rcp = ctx.enter_context(tc.tile_pool(name="rcp", bufs=4))
        xsqp = ctx.enter_context(tc.tile_pool(name="xsqp", bufs=2))
        ring_pool = ctx.enter_context(
            tc.tile_pool(name="ring_pool", bufs=3, space="PSUM"))
        flex_pool = ctx.enter_context(
            tc.tile_pool(name="flex_pool", bufs=1, space="PSUM"))

        # ---------- input / weight DMAs (stats-critical first) ----------
        x_sb = [xp.tile([128, S], F32, tag=f"x{cc}", name=f"x{cc}")
                for cc in range(NCC)]
        for cc in range(NCC):
            nc.sync.dma_start(x_sb[cc][:], x_d[128 * cc:128 * (cc + 1), :])
        gsum_sb = gnp.tile([C // NCC, NG * NCC], F32R)
        for cc in range(NCC):
            nc.sync.dma_start(gsum_sb[:, NG * cc:NG * (cc + 1)],
                              gsum_d[128 * cc:128 * (cc + 1), :])
        w4_sb = const.tile([128, NCC], F32)
        nc.sync.dma_start(w4_sb[:], w4_d[:])
        b4_sb = const.tile([128, NCC], F32)
        nc.sync.dma_start(b4_sb[:], b4_d[:])
        gexp_sb = const.tile([NG, C], F32R)
        nc.sync.dma_start(gexp_sb[:], gexp_d[:])
        qb_sb = const.tile([128, 12], F32)
        nc.sync.dma_start(qb_sb[:], qb_d[:])

        qkvw_sb = [wqp.tile([128, 3 * C], BF16, tag=f"w{cc}", name=f"w{cc}")
                   for cc in range(NCC)]
        for cc in range(NCC):
            nc.sync.dma_start(qkvw_sb[cc][:],
                              qkvw_d[128 * cc:128 * (cc + 1), :])
        vb_sb = const.tile([128, C], F32)
        nc.sync.dma_start(vb_sb[:], vb_d[:])
        pb_sb = const.tile([128, NCC], F32)
        nc.sync.dma_start(pb_sb[:], pb_d[:])
        pw_sb = [pwp.tile([128, C], BF16, tag=f"pw{cc}", name=f"pw{cc}")
                 for cc in range(NCC)]
        for cc in range(NCC):
            nc.sync.dma_start(pw_sb[cc][:], projw_d[128 * cc:128 * (cc + 1), :])

        # ---------- SBUF working tiles ----------
        q_sb = [qp.tile([128, S], BF16, tag=f"q{cc}", name=f"q{cc}")
                for cc in range(NH // 2)]
        k_sb = [kp.tile([128, S], BF16, tag=f"k{cc}", name=f"k{cc}")
                for cc in range(NH // 2)]
        # [64 v-channels | 1.0 | pad] per head block (fp8, DoubleRow layout
        # [key%128, kc, head*VHB + ch]): the ones column turns the attn@V
        # matmul (M=65) into attn@V plus the softmax denominator row.
        vt3 = vp.tile([128, NSC, VHB * NH], BF16, tag="vt3")
        an_sb = [anp.tile([128, S], BF16, tag=f"an{cc}", name=f"an{cc}")
                 for cc in range(NCC)]
        xn_sb = [xnp.tile([128, S], BF16, tag=f"xn{cc}", name=f"xn{cc}")
                 for cc in range(NCC)]
        vt3h = vt3[:].rearrange("p s (h u) -> p s h u", u=VHB)
        nc.vector.memset(vt3h[:, :, :, 64:65], 1.0)
        expb_sb = const.tile([128, 1], F32)
        nc.vector.memset(expb_sb[:], EXPB)

        # PSUM: 6-bank score ring (3 rotating region tiles) + 2-bank flex.
        flex = flex_pool.tile([128, S], F32, tag="flex")

        # load the ln/exp ACT table set while the input DMAs run
        warm = gnp.tile([1, 1], F32)
        nc.vector.memset(warm[:], 1.0)
        nc.scalar.activation(out=warm[:], in_=warm[:], func=AF.Ln,
                             bias=warm[:], scale=1.0)

        # ================= GroupNorm ================
        # per-channel sum (DVE accumulate) and sum of squares (ACT Square
        # accumulate); a tiny f32r matmul against the group map then does
        # the cross-partition group reduction.
        s12 = gnp.tile([128, 2 * NCC], F32)
        for cc in range(NCC):
            scr = xsqp.tile([128, S], BF16, tag="scr")
            nc.vector.scalar_tensor_tensor(
                out=scr[:], in0=x_sb[cc][:], scalar=1.0, in1=x_sb[cc][:],
                op0=AL.mult, op1=AL.bypass,
                accum_out=s12[:, 2 * cc:2 * cc + 1])
            scr2 = xsqp.tile([128, S], BF16, tag="scr2")
            nc.scalar.activation(
                out=scr2[:], in_=x_sb[cc][:], func=AF.Square,
                accum_out=s12[:, 2 * cc + 1:2 * cc + 2])
        s12r = gnp.tile([128, 2 * NCC], F32R)
        nc.vector.tensor_copy(s12r[:], s12[:])
        ps_g = flex[0:NG, 0:2]
        for cc in range(NCC):
            nc.tensor.matmul(
                ps_g, gsum_sb[:, NG * cc:NG * (cc + 1)],
                s12r[:, 2 * cc:2 * cc + 2],
                start=(cc == 0), stop=(cc == NCC - 1))
        inv_n = 1.0 / (GS * S)
        mean_g = gnp.tile([NG, 1], F32)
        nc.vector.tensor_scalar(out=mean_g[:], in0=ps_g[:, 0:1],
                                scalar1=inv_n,
                                scalar2=None, op0=AL.mult)
        ex2 = gnp.tile([NG, 1], F32)
        nc.vector.tensor_scalar(out=ex2[:], in0=ps_g[:, 1:2],
                                scalar1=inv_n,
                                scalar2=None, op0=AL.mult)
        var_g = gnp.tile([NG, 1], F32)
        # var = E[x^2] - mean^2
        nc.vector.scalar_tensor_tensor(
            out=var_g[:], in0=mean_g[:], scalar=-1.0, in1=mean_g[:],
            op0=AL.mult, op1=AL.mult)
        nc.vector.tensor_tensor(out=var_g[:], in0=ex2[:], in1=var_g[:],
                                op=AL.add)
        # rstd = exp(-0.5 * ln(var + eps)); ln+exp share one ACT table set
        eps_sb = gnp.tile([NG, 1], F32)
        nc.vector.memset(eps_sb[:], EPS)
        lnv = gnp.tile([NG, 1], F32)
        nc.scalar.activation(out=lnv[:], in_=var_g[:], func=AF.Ln,
                             bias=eps_sb[:], scale=1.0)
        # stats_r[:, 0] = rstd, stats_r[:, 1] = mean  (N=2 matmul rhs)
        stats_r = gnp.tile([NG, 2], F32R)
        nc.scalar.activation(out=stats_r[:, 0:1], in_=lnv[:], func=AF.Exp,
                             bias=0.0, scale=-0.5)
        nc.vector.tensor_copy(stats_r[:, 1:2], mean_g[:])

        # per-channel rstd/mean via tiny matmuls against the group map
        rstd_c = gnp.tile([128, NCC], F32)
        mean_c = gnp.tile([128, NCC], F32)
        for cc in range(NCC):
            ps_a = flex[:, 4 + 2 * cc:6 + 2 * cc]
            nc.tensor.matmul(ps_a,
                             gexp_sb[:, 128 * cc:128 * (cc + 1)],
                             stats_r[:], start=True, stop=True)
            nc.vector.tensor_copy(rstd_c[:, cc:cc + 1], ps_a[:, 0:1])
            nc.vector.tensor_copy(mean_c[:, cc:cc + 1], ps_a[:, 1:2])
        alpha = gnp.tile([128, NCC], F32)
        nc.vector.tensor_tensor(out=alpha[:], in0=rstd_c[:], in1=w4_sb[:],
                                op=AL.mult)
        beta = gnp.tile([128, NCC], F32)
        nc.vector.tensor_tensor(out=beta[:], in0=alpha[:], in1=mean_c[:],
                                op=AL.mult)
        nc.vector.tensor_tensor(out=beta[:], in0=b4_sb[:], in1=beta[:],
                                op=AL.subtract)

        # ---------- GN apply: split ACT / DVE ----------
        for cc in range(NCC):
            if cc < 2:
                nc.scalar.activation(
                    out=xn_sb[cc][:], in_=x_sb[cc][:], func=AF.Identity,
                    bias=beta[:, cc:cc + 1], scale=alpha[:, cc:cc + 1])
            else:
                nc.vector.tensor_scalar(
                    out=xn_sb[cc][:], in0=x_sb[cc][:],
                    scalar1=alpha[:, cc:cc + 1], scalar2=beta[:, cc:cc + 1],
                    op0=AL.mult, op1=AL.add)

        # ---------- Q0 / K0 (in ring-pool rotations, evicted pre-scores) --
        for dst, woff, boff in ((q_sb[0], 0, 0), (k_sb[0], 512, 4)):
            ps_qk = ring_pool.tile([128, S], F32, tag="sc", name="ps_qk")
            for cc in range(NCC):
                for hq in range(2):
                    nc.tensor.matmul(ps_qk[:, 512 * hq:512 * (hq + 1)],
                                     qkvw_sb[cc][:, woff:woff + 128],
                                     xn_sb[cc][:, 512 * hq:512 * (hq + 1)],
                                     start=(cc == 0), stop=(cc == NCC - 1))
            nc.vector.tensor_scalar(out=dst[:], in0=ps_qk[:],
                                    scalar1=qb_sb[:, boff:boff + 1],
                                    scalar2=None, op0=AL.add)

        # ================= attention ================
        # Emission helpers. Iteration it = (pair p = it>>1, qn = it&1).
        sc_tiles = {}
        ep_tiles = {}

        def emit_scores(it, kc):
            p, qn = it >> 1, it & 1
            g = 8 * it + kc
            sc_t = ring_pool.tile([128, S], F32, tag="sc", name=f"sc{g}")
            sc_tiles[g] = sc_t
            nc.tensor.matmul(
                sc_t[:, 0:512],
                k_sb[p][0:64, 128 * kc:128 * (kc + 1)],
                q_sb[p][0:64, 512 * qn:512 * (qn + 1)],
                start=True, stop=True, tile_position=(0, 0))
            nc.tensor.matmul(
                sc_t[:, 512:1024],
                k_sb[p][64:128, 128 * kc:128 * (kc + 1)],
                q_sb[p][64:128, 512 * qn:512 * (qn + 1)],
                start=True, stop=True, tile_position=(64, 0))

        def emit_exps(it, kc):
            g = 8 * it + kc
            ep_t = xep.tile([128, S], BF16, tag="ep", name=f"ep{g}")
            ep_tiles[g] = ep_t
            nc.scalar.activation(out=ep_t[:],
                                 in_=sc_tiles.pop(g)[:],
                                 func=AF.Exp, bias=expb_sb[:], scale=SCALE)

        def emit_attnv(av, it, kc):
            p = it >> 1
            g = 8 * it + kc
            ep_t = ep_tiles.pop(g)
            for h in range(2):
                hh = 2 * p + h
                nc.tensor.matmul(
                    av[0:65, 512 * h:512 * (h + 1)],
                    vt3[:, kc, VHB * hh:VHB * hh + 65],
                    ep_t[:, 512 * h:512 * (h + 1)],
                    start=(kc == 0), stop=(kc == NSC - 1))

        def emit_vt(sc):
            psv = flex[:, 512 * (sc % 2):512 * (sc % 2) + 512]
            for cc in range(NCC):
                nc.tensor.matmul(
                    psv,
                    xn_sb[cc][:, 128 * sc:128 * (sc + 1)],
                    qkvw_sb[cc][:, 1024:1536],
                    start=(cc == 0), stop=(cc == NCC - 1))
            nc.vector.tensor_tensor(
                out=vt3h[:, sc, :, 0:64],
                in0=psv.rearrange("p (h u) -> p h u", u=64),
                in1=vb_sb[:].rearrange("p (h u) -> p h u", u=64),
                op=AL.add)

        def emit_blob(tgt_pair, is_k):
            woff = 512 + 128 * tgt_pair if is_k else 128 * tgt_pair
            for cc in range(NCC):
                for hq in range(2):
                    nc.tensor.matmul(flex[:, 512 * hq:512 * (hq + 1)],
                                     qkvw_sb[cc][:, woff:woff + 128],
                                     xn_sb[cc][:, 512 * hq:512 * (hq + 1)],
                                     start=(cc == 0), stop=(cc == NCC - 1))
            dst = k_sb[tgt_pair] if is_k else q_sb[tgt_pair]
            boff = 4 + tgt_pair if is_k else tgt_pair
            nc.vector.tensor_scalar(out=dst[:], in0=flex[:, :],
                                    scalar1=qb_sb[:, boff:boff + 1],
                                    scalar2=None, op0=AL.add)

        # finish is split: emit_recip evicts attn@V + launches the
        # denominator-reciprocal DMA round trip; emit_norm (emitted two
        # iterations later, once the broadcast has surely landed) does the
        # softmax-normalize multiplies. This keeps the DMA latency off the
        # in-order DVE/PE queues.
        norm_state = {}

        def emit_recip(av, it):
            p, qn = it >> 1, it & 1
            raw = rcp.tile([65, S], F32, tag="raw")
            nc.vector.tensor_copy(raw[:], av[0:65, :])
            d128 = rcp.tile([128, 8], F32, tag="d128")
            nc.sync.dma_start(d128[:], raw[64:65, :])
            r128 = rcp.tile([128, 8], F32, tag="r128")
            rscr = rcp.tile([128, 8], F32, tag="rscr")
            nc.vector.reciprocal_approx_accurate(
                out=r128[:], in_=d128[:], scratch=rscr[:])
            r128v = recip_d[p][qn].rearrange("h (x f) -> (h x) f", f=8)
            nc.sync.dma_start(r128v, r128[:])
            rb = rcp.tile([64, S], F32, tag="rb")
            rsrc = recip_d[p][qn].rearrange("h f -> (h f)")  # [1024]
            rsrc_b = bass.AP(tensor=rsrc.tensor,
                             offset=rsrc.offset,
                             ap=[[0, 64], list(rsrc.ap[0])])
            nc.sync.dma_start(rb[:], rsrc_b)
            norm_state[it] = (raw, rb)

        def emit_norm(it):
            p, qn = it >> 1, it & 1
            raw, rb = norm_state.pop(it)
            nc.vector.tensor_tensor(
                out=an_sb[p][0:64, 512 * qn:512 * (qn + 1)],
                in0=raw[0:64, 0:512], in1=rb[:, 0:512],
                op=AL.mult)
            nc.vector.tensor_tensor(
                out=an_sb[p][64:128, 512 * qn:512 * (qn + 1)],
                in0=raw[0:64, 512:1024], in1=rb[:, 512:1024],
                op=AL.mult)

        # blobs delivered after finishing iteration it: [(target pair, is_k)]
        # finish(2P-2) is emitted at the end of iteration 2P-1, just before
        # pair P's scores start in iteration 2P.
        blob_after = {0: [(1, False), (1, True)],
                      2: [(2, False), (2, True)],
                      4: [(3, False), (3, True)]}


        av_tiles = {}
        for it in range(NIT):
            if it > 0:
                av_tiles[it - 1] = flex[:, :]
            for kc in range(NSC):
                emit_scores(it, kc)
                emit_exps(it, kc)
                if it == 0:
                    # V^T rides in iteration 0 on the flex banks
                    emit_vt(kc)
                else:
                    emit_attnv(av_tiles[it - 1], it - 1, kc)
            if it == 0:
                continue
            # finish iteration it-1: denominators, normalize, then the
            # flex banks host the next Q/K blob.
            emit_recip(av_tiles[it - 1], it - 1)
            for blob in blob_after.get(it - 1, ()):
                emit_blob(*blob)
            if it - 3 >= 0:
                emit_norm(it - 3)
        # last iteration's attn@V (runs as its exps land), then proj
        # partials for the already-normalized pairs overlap the recip tail.
        av_tiles[NIT - 1] = flex[:, :]
        for kc in range(NSC):
            emit_attnv(av_tiles[NIT - 1], NIT - 1, kc)
        emit_recip(av_tiles[NIT - 1], NIT - 1)
        emit_norm(NIT - 3)
        # ================= proj + bias + residual ================
        # oc 0-2 in ring-pool rotations (banks free as the last exps drain),
        # oc 3 on flex (frees after the last raw eviction).
        proj_ps = [ring_pool.tile([128, S], F32, tag="sc", name=f"pso{oc}")
                   for oc in range(3)] + [flex[:, :]]
        # partials over the already-normalized pairs (cc 0-2) overlap the
        # last reciprocal's DMA round trip; oc3 (flex) starts once the raw
        # eviction has freed the flex banks.
        for cc in range(3):
            for oc in range(NCC):
                tgt = proj_ps[oc]
                for hq in range(2):
                    nc.tensor.matmul(
                        tgt[:, 512 * hq:512 * (hq + 1)],
                        pw_sb[cc][:, 128 * oc:128 * (oc + 1)],
                        an_sb[cc][:, 512 * hq:512 * (hq + 1)],
                        start=(cc == 0), stop=False)
        emit_norm(NIT - 2)
        emit_norm(NIT - 1)
        for oc in range(NCC):
            for hq in range(2):
                nc.tensor.matmul(
                    proj_ps[oc][:, 512 * hq:512 * (hq + 1)],
                    pw_sb[3][:, 128 * oc:128 * (oc + 1)],
                    an_sb[3][:, 512 * hq:512 * (hq + 1)],
                    start=False, stop=True)
        ps_o = proj_ps
        for oc in range(NCC):
            out_t = outp.tile([128, S], F32, tag="out")
            if oc < 2:
                nc.vector.scalar_tensor_tensor(
                    out=out_t[:], in0=ps_o[oc],
                    scalar=pb_sb[:, oc:oc + 1], in1=x_sb[oc][:],
                    op0=AL.add, op1=AL.add)
            else:
                tmp_t = outp.tile([128, S], F32, tag="tmp")
                nc.scalar.activation(out=tmp_t[:], in_=ps_o[oc],
                                     func=AF.Identity,
                                     bias=pb_sb[:, oc:oc + 1], scale=1.0)
                nc.vector.tensor_tensor(out=out_t[:], in0=tmp_t[:],
                                        in1=x_sb[oc][:],
                                        op=AL.add)
            nc.sync.dma_start(y_d[128 * oc:128 * (oc + 1), :], out_t[:])

    nc.finalize()
    return nc


_NC_CACHE = None


def _get_nc():
    global _NC_CACHE
    if _NC_CACHE is None:
        _NC_CACHE = build_nc()
    return _NC_CACHE


def make_in_maps(X, norm_w, norm_b, qkv_w, qkv_b, proj_w, proj_b):
    X = np.asarray(X, dtype=np.float32)
    norm_w = np.asarray(norm_w, dtype=np.float32)
    norm_b = np.asarray(norm_b, dtype=np.float32)
    qkv_w = np.asarray(qkv_w, dtype=np.float32)
    qkv_b = np.asarray(qkv_b, dtype=np.float32)
    proj_w = np.asarray(proj_w, dtype=np.float32)
    proj_b = np.asarray(proj_b, dtype=np.float32)

    qkv_wT = np.ascontiguousarray(qkv_w.T).astype(ml_dtypes.bfloat16)
    proj_wT = np.ascontiguousarray(proj_w.T).astype(ml_dtypes.bfloat16)
    gsum = np.zeros((C, NG), np.float32)
    gsum[np.arange(C), np.arange(C) // GS] = 1.0
    gexpT = np.ascontiguousarray(gsum.T)                      # [32, 512]
    w4 = np.ascontiguousarray(norm_w.reshape(NCC, 128).T)     # [128, 4]
    b4 = np.ascontiguousarray(norm_b.reshape(NCC, 128).T)
    qb12 = np.ascontiguousarray(qkv_b.reshape(12, 128).T)     # [128, 12]
    vb_bcast = np.ascontiguousarray(
        np.broadcast_to(qkv_b[2 * C:3 * C], (128, C)))        # [128, 512]
    pb4 = np.ascontiguousarray(proj_b.reshape(NCC, 128).T)

    shared = {
        "qkv_wT": qkv_wT, "proj_wT": proj_wT, "gsum": gsum, "gexpT": gexpT,
        "norm_w4": w4, "norm_b4": b4, "qkv_b12": qb12, "vb_bcast": vb_bcast,
        "proj_b4": pb4,
    }
    in_maps = []
    for b in range(B):
        m = dict(shared)
        m["x"] = np.ascontiguousarray(X[b].reshape(C, S))
        in_maps.append(m)
    return in_maps


def kernel(X, norm_w, norm_b, qkv_w, qkv_b, proj_w, proj_b):
    nc = _get_nc()
    in_maps = make_in_maps(X, norm_w, norm_b, qkv_w, qkv_b, proj_w, proj_b)
    res = run_bass_kernel_spmd(nc, in_maps, core_ids=list(range(B)))
    out = np.stack([res.results[b]["y"].reshape(C, H, W) for b in range(B)])
    return out.astype(np.float32)
